# revision 18
# baseline (speedup 1.0000x reference)
"""DGCNN semantic-segmentation kernel for 8x Trainium2 NeuronCores.

Strategy: data-parallel over batch. B=4 samples; core c processes sample c%4
end-to-end (cores 4-7 duplicate work so one SPMD program runs everywhere);
host takes outputs from cores 0-3. Everything for one sample stays on one
core: no cross-core communication.

Per-sample pipeline (all on device):
  3x EdgeConv blocks: kNN (fp16 hi/lo split K=13 distance matmul, ~1e-6
  accurate) -> top-20 selection (pack 8-bit chunk-local index into low
  mantissa bits, top-8 per 256-chunk via DVE max8, refine via
  max8/match_replace, recover columns arithmetically) -> streamed pair-gather
  via GPSIMD ap_gather (2 neighbors per instruction: gather cost is
  source-size bound, so pairs halve it) -> conv1 as packed block-diagonal
  float32r matmuls (1 cyc/row vs 4 for fp32) -> instance-norm stats via
  accum_out -> second conv as block-diagonal f32r matmul streamed with
  running max over k. Then the global head (w6..w9) with in1d norms, bf16
  matmuls, ACT-copy-with-accum for sums and DVE squares for sumsq.

Inter-block layout: packed [128, 2048] - partition p<64 holds channel p
for points n<2048, partition 64+p holds channel p for n>=2048.
"""

import sys

if "/opt/trn_rl_repo" not in sys.path:
    sys.path.insert(0, "/opt/trn_rl_repo")

import numpy as np

N = 4096
NT = 32          # row tiles of 128 for the distance/selection loop
HB = 2048
KNN = 20
NK = KNN * HB    # free size of the (virtual) packed h tensor per partition
CHUNK = 256
NEG = -3.0e38

# "all": conv1/conv2 matmuls in float32r (4x PE rate, ~2e-4 rounding);
# "none": same block-diagonal structure in plain fp32 (exact, 2x PE rate)
F32R_MODE = "none"

_CACHE = {}


def _build_program():
    import concourse.bacc as bacc
    import concourse.tile as tile
    from concourse import mybir
    from contextlib import ExitStack

    F32 = mybir.dt.float32
    F32R = mybir.dt.float32r
    F16 = mybir.dt.float16
    BF16 = mybir.dt.bfloat16
    U32 = mybir.dt.uint32
    U16 = mybir.dt.uint16
    I16 = mybir.dt.int16
    AF = mybir.ActivationFunctionType
    ALU = mybir.AluOpType
    AX = mybir.AxisListType

    nc = bacc.Bacc("TRN2", target_bir_lowering=False, debug=False, num_devices=8)

    def din(name, shape, dt=F32):
        return nc.dram_tensor(name, shape, dt, kind="ExternalInput").ap()

    xt_d = din("xt", [4, N])
    # block-diagonal conv1/conv2 weights, one [128,128] block per use
    waD_d = [[din(f"waD{i}_h{h}", [128, 128]) for h in range(2)] for i in range(3)]
    wbD_d = [din(f"wbD{i}", [128, 128]) for i in range(3)]
    wcD_d = [din(f"wcD{i}", [128, 128]) for i in range(2)]
    wcT_d = [din("w2T", [64, 64]), din("w4T", [64, 64])]
    w6T_d = [din(f"w6T_{k}", [64, 1024], BF16) for k in range(3)]
    w6T12_d = din("w6T12", [128, 1024], BF16)
    w7xT12_d = din("w7xT12", [128, 512], BF16)
    w7gT_d = [din(f"w7gT_{k}", [128, 512], BF16) for k in range(8)]
    w7xT_d = [din(f"w7xT_{k}", [64, 512], BF16) for k in range(3)]
    w8T_d = [din(f"w8T_{k}", [128, 256], BF16) for k in range(4)]
    w9T_d = [din(f"w9T_{k}", [128, 2], BF16) for k in range(2)]
    iota512_d = din("iota512", [128, CHUNK], U32)
    maskc_d = din("maskc", [128, 1], U32)
    magic_d = din("magic", [128, 1], U32)
    id2_d = din("id2", [2, 2])

    out_d = nc.dram_tensor("out", [N, 2], F32, kind="ExternalOutput").ap()

    with tile.TileContext(nc) as tc, ExitStack() as ctx:
        wpool = ctx.enter_context(tc.tile_pool(name="wpool", bufs=1))
        xpool = ctx.enter_context(tc.tile_pool(name="xpool", bufs=1))
        stpool = ctx.enter_context(tc.tile_pool(name="stpool", bufs=1))
        pp = ctx.enter_context(tc.tile_pool(name="pp", bufs=2, space="PSUM"))
        ectx = ExitStack()
        abpool = ectx.enter_context(tc.tile_pool(name="abpool", bufs=1))
        gpool = ectx.enter_context(tc.tile_pool(name="gpool", bufs=1))
        dpool = ectx.enter_context(tc.tile_pool(name="dpool", bufs=2))
        selpool = ectx.enter_context(tc.tile_pool(name="selpool", bufs=4))
        idxpool = ectx.enter_context(tc.tile_pool(name="idxpool", bufs=1))
        chpool = ectx.enter_context(tc.tile_pool(name="chpool", bufs=1))

        def load(pool, ap_d, shape, dt=F32, dup64=False):
            rows = shape[0]
            tshape = [128, shape[1]] if dup64 else shape
            t = pool.tile(tshape, dt, tag=ap_d.tensor.name, name=ap_d.tensor.name + "_sb")
            nc.sync.dma_start(t[0:rows, :], ap_d)
            if dup64:
                nc.sync.dma_start(t[64:64 + rows, :], ap_d)
            return t

        MMDT = F32R if F32R_MODE == "all" else F32

        def load_f32r(ap_d):
            """DMA an f32 [128,128] block weight; round to f32r via DVE copy."""
            if MMDT is F32:
                t = wpool.tile([128, 128], F32, tag=ap_d.tensor.name,
                               name=ap_d.tensor.name + "_sb")
                nc.sync.dma_start(t[:], ap_d)
                return t
            st = wpool.tile([128, 128], F32, tag="wstage", bufs=2,
                            name=ap_d.tensor.name + "_st")
            nc.sync.dma_start(st[:], ap_d)
            t = wpool.tile([128, 128], F32R, tag=ap_d.tensor.name,
                           name=ap_d.tensor.name + "_sb")
            nc.vector.tensor_copy(t[:], st[:])
            return t

        # ---- early loads: input + block-structure constants ----
        xt_sb = gpool.tile([4, N], F32, tag="ya_dup", name="xt_sb")
        nc.sync.dma_start(xt_sb[:], xt_d)
        iota512 = load(wpool, iota512_d, [128, CHUNK], U32)
        maskc = load(wpool, maskc_d, [128, 1], U32)
        magic = load(wpool, magic_d, [128, 1], U32)
        waD = [[load_f32r(waD_d[0][h]) for h in range(2)], None, None]
        wbD = [load_f32r(wbD_d[0]), None, None]
        wcD = [load_f32r(wcD_d[0]), None]
        wcT = [load(wpool, wcT_d[0], [64, 64], dup64=True),
               load(wpool, wcT_d[1], [64, 64], dup64=True)]

        x_p = [xpool.tile([128, HB], MMDT, tag=f"x{i}_p", name=f"x{i}_p") for i in range(4)]
        # zero-fill first: block-diagonal matmuls stream all 128 partitions, so
        # unused rows must be finite (0 * w = 0, but NaN * 0 = NaN)
        nc.gpsimd.memset(x_p[0][:].bitcast(F32), 0.0)
        nc.vector.tensor_copy(x_p[0][0:4, :], xt_sb[:, 0:HB])
        nc.vector.tensor_copy(x_p[0][64:68, :], xt_sb[:, HB:N])

        HEADW = {}

        def _head_loads():
            for i in (1, 2):
                waD[i] = [load_f32r(waD_d[i][h]) for h in range(2)]
                wbD[i] = load_f32r(wbD_d[i])
            wcD[1] = load_f32r(wcD_d[1])
            HEADW["w6T"] = [None, None, load(wpool, w6T_d[2], [64, 1024], BF16, dup64=True)]
            HEADW["w6T12"] = load(wpool, w6T12_d, [128, 1024], BF16)
            HEADW["w7xT12"] = load(wpool, w7xT12_d, [128, 512], BF16)
            HEADW["w7gT"] = [load(wpool, a, [128, 512], BF16) for a in w7gT_d]
            HEADW["w7xT"] = [None, None, load(wpool, w7xT_d[2], [64, 512], BF16, dup64=True)]
            HEADW["w8T"] = [load(wpool, a, [128, 256], BF16) for a in w8T_d]
            HEADW["w9T"] = [load(wpool, a, [128, 2], BF16) for a in w9T_d]
            HEADW["id2"] = load(wpool, id2_d, [2, 2])

        def small(tag, shape=(128, 1), dt=F32):
            return stpool.tile(list(shape), dt, tag=tag, name=tag)

        def ts(out, in0, s1, op0, s2=None, op1=None):
            if op1 is None:
                nc.vector.tensor_scalar(out, in0, s1, None, op0)
            else:
                nc.vector.tensor_scalar(out, in0, s1, s2, op0, op1)

        def rsqrt_inplace(y, t_in, rows):
            b = small("rs_b", (rows, 1), U32)
            ts(b[:], t_in.bitcast(U32), 1, ALU.logical_shift_right)
            nc.vector.tensor_tensor(y.bitcast(U32), magic[0:rows, :], b[:], ALU.subtract)
            for _ in range(2):
                u = small("rs_u", (rows, 1))
                nc.vector.tensor_tensor(u[:], y, y, ALU.mult)
                nc.vector.tensor_tensor(u[:], u[:], t_in, ALU.mult)
                ts(u[:], u[:], -0.5, ALU.mult, 1.5, ALU.add)
                nc.vector.tensor_tensor(y, y, u[:], ALU.mult)

        def scale_bias_from_mv(mv):
            """mv [128,2] per-partition (mean, var); rows p/p+64 are halves of one
            channel. Returns s128, b128 [128,1] with normalized = s*x + b."""
            mvb = small("st_mvb", (64, 2))
            nc.sync.dma_start(mvb[:], mv[64:128, :])
            m = small("st_m", (64, 1)); v = small("st_v", (64, 1))
            dm = small("st_dm", (64, 1))
            nc.vector.tensor_tensor(m[:], mv[0:64, 0:1], mvb[:, 0:1], ALU.add)
            ts(m[:], m[:], 0.5, ALU.mult)
            nc.vector.tensor_tensor(v[:], mv[0:64, 1:2], mvb[:, 1:2], ALU.add)
            nc.vector.tensor_tensor(dm[:], mv[0:64, 0:1], mvb[:, 0:1], ALU.subtract)
            nc.vector.tensor_tensor(dm[:], dm[:], dm[:], ALU.mult)
            ts(v[:], v[:], 0.5, ALU.mult)
            ts(dm[:], dm[:], 0.25, ALU.mult)
            nc.vector.tensor_tensor(v[:], v[:], dm[:], ALU.add)
            ts(v[:], v[:], 1e-5, ALU.add)
            s = small("st_s", (64, 1))
            rsqrt_inplace(s[:], v[:], 64)
            bb = small("st_bb", (64, 1))
            nc.vector.tensor_tensor(bb[:], m[:], s[:], ALU.mult)
            ts(bb[:], bb[:], -1.0, ALU.mult)
            s128 = small("st_s128"); b128 = small("st_b128")
            nc.vector.tensor_copy(s128[0:64, :], s[:])
            nc.vector.tensor_copy(b128[0:64, :], bb[:])
            nc.sync.dma_start(s128[64:128, :], s[:])
            nc.sync.dma_start(b128[64:128, :], bb[:])
            return s128, b128

        def mv_from_sums(ssum, ssq, count):
            """[128, w] partial sums -> mv [128, 2] (mean, var per partition)."""
            mv = small("sm_mv", (128, 2))
            nc.vector.tensor_reduce(mv[:, 0:1], ssum[:], axis=AX.X, op=ALU.add)
            nc.vector.tensor_reduce(mv[:, 1:2], ssq[:], axis=AX.X, op=ALU.add)
            ts(mv[:, 0:1], mv[:, 0:1], 1.0 / count, ALU.mult)
            ts(mv[:, 1:2], mv[:, 1:2], 1.0 / count, ALU.mult)
            m2 = small("sm_m2")
            nc.vector.tensor_tensor(m2[:], mv[:, 0:1], mv[:, 0:1], ALU.mult)
            nc.vector.tensor_tensor(mv[:, 1:2], mv[:, 1:2], m2[:], ALU.subtract)
            return mv

        # ---------------- EdgeConv block ----------------
        def edge_block(bi, xin_p, C):
            has_conv2 = bi < 2
            xin_f = xin_p[:].bitcast(F32)

            # distance operand prep: pieces computed at partition base 0 (ACT
            # alignment rule), assembled into aT/bT via DMAs.
            sq = dpool.tile([128, HB], F32, tag="dpk", name="sq")
            nc.scalar.activation(sq[0:3, :], xin_f[0:3, :], AF.Square)
            nc.scalar.activation(sq[64:67, :], xin_f[64:67, :], AF.Square)
            ones3 = abpool.tile([128, 1], F32, tag="ones3")
            nc.gpsimd.memset(ones3[:], 1.0)

            src4 = gpool.tile([4, N], F32, tag="ya_dup", name="src4")
            for h in range(2):
                psx = pp.tile([128, HB], F32, tag="pp")
                for j in range(4):
                    nc.tensor.matmul(
                        psx[0:1, 512 * j:512 * (j + 1)],
                        ones3[64 * h:64 * h + 3, 0:1],
                        sq[64 * h:64 * h + 3, 512 * j:512 * (j + 1)])
                nc.scalar.copy(src4[0:1, HB * h:HB * (h + 1)], psx[0:1, 0:HB])
                nc.sync.dma_start(src4[1:4, HB * h:HB * (h + 1)], xin_f[64 * h:64 * h + 3, :])

            hi4 = gpool.tile([4, N], F16, tag="M", name="hi4")
            lo4 = gpool.tile([4, N], F16, tag="yb_p", name="lo4")
            nc.scalar.copy(hi4[:], src4[:])
            nc.vector.scalar_tensor_tensor(
                lo4[:], hi4[:], -1.0, src4[:], ALU.mult, ALU.add)
            nh4 = dpool.tile([4, N], F16, tag="dpk", name="nh4")
            nl4 = dpool.tile([4, N], F16, tag="dpk", name="nl4")
            nc.scalar.mul(nh4[:], hi4[:], -1.0)
            nc.scalar.mul(nl4[:], lo4[:], -1.0)

            # aT rows: [1, 1, -xxh, -xxl, 2ph(3), 2ph(3), 2pl(3)]
            # bT rows: [-xxh, -xxl, 1, 1, ph(3), pl(3), ph(3)]
            aT = abpool.tile([16, N], F16, tag="aT")
            bT = abpool.tile([16, N], F16, tag="bT")
            nc.gpsimd.memset(aT[0:2, :], 1.0)
            nc.sync.dma_start(bT[2:4, :], aT[0:2, :])
            nc.sync.dma_start(aT[2:3, :], nh4[0:1, :])
            nc.sync.dma_start(aT[3:4, :], nl4[0:1, :])
            nc.sync.dma_start(bT[0:1, :], nh4[0:1, :])
            nc.sync.dma_start(bT[1:2, :], nl4[0:1, :])
            h2x = dpool.tile([4, N], F16, tag="dpk", name="h2x")
            l2x = dpool.tile([4, N], F16, tag="dpk", name="l2x")
            ts(h2x[:], hi4[:], 2.0, ALU.mult)
            ts(l2x[:], lo4[:], 2.0, ALU.mult)
            nc.sync.dma_start(aT[4:7, :], h2x[1:4, :])
            nc.sync.dma_start(aT[7:10, :], h2x[1:4, :])
            nc.sync.dma_start(aT[10:13, :], l2x[1:4, :])
            nc.sync.dma_start(bT[4:7, :], hi4[1:4, :])
            nc.sync.dma_start(bT[7:10, :], lo4[1:4, :])
            nc.sync.dma_start(bT[10:13, :], hi4[1:4, :])

            # ya (duplicated to both partition halves, full n) and yb (packed)
            # via packed block-diagonal matmuls. Issued from inside the
            # selection loop: they are only needed by the gather passes, and
            # issuing them here keeps them off the pre-selection critical path
            # (PE has slack while the DVE runs selection).
            ya_dup = gpool.tile([128, N], F32, tag="ya_dup")
            yb_p = gpool.tile([128, HB], F32, tag="yb_p")

            def emit_conv1():
                for h in range(2):
                    psy = pp.tile([128, HB], F32, tag="pp")
                    for j in range(4):
                        sl = slice(512 * j, 512 * (j + 1))
                        nc.tensor.matmul(psy[:, sl], waD[bi][h][:], xin_p[:, sl])
                    nc.scalar.copy(ya_dup[:, HB * h:HB * (h + 1)], psy[:, 0:HB])
                psb = pp.tile([128, HB], F32, tag="pp")
                for j in range(4):
                    sl = slice(512 * j, 512 * (j + 1))
                    nc.tensor.matmul(psb[:, sl], wbD[bi][:], xin_p[:, sl])
                nc.scalar.copy(
                    yb_p[:].rearrange("p (g t q) -> p t g q", g=8, t=16, q=16),
                    psb[:, 0:HB].rearrange("p (t g q) -> p t g q", t=16, g=8, q=16))

            # ---- distance + selection ----
            colbuf = idxpool.tile([128, 768], U16, tag="colbuf")
            posall = idxpool.tile([128, 768], U16, tag="posall")
            v24all = idxpool.tile([128, 768], F32, tag="v24all")
            # wrapped gather index lists (per half): position i = j*2048+n_loc;
            # stored wrapped-16: partition 16k + (i%16), free i//16
            wrapped = idxpool.tile([128, 2560], U16, tag="wrapped")

            def emit_half_idx(h):
                # column arithmetic for half h's tiles, then reformat DMAs.
                # Issued right after the half's last selection tile so the DMA
                # chain hides under the other half's selection.
                hsl = slice(384 * h, 384 * (h + 1))
                locb = idxpool.tile([128, 384], U32, tag="locb")
                ts(locb[:], v24all[:, hsl].bitcast(U32), 255, ALU.bitwise_and)
                loc16b = idxpool.tile([128, 384], U16, tag="loc16b")
                nc.vector.tensor_copy(loc16b[:], locb[:])
                ts(posall[:, hsl], posall[:, hsl], 3, ALU.logical_shift_right)
                ts(posall[:, hsl], posall[:, hsl], 8, ALU.logical_shift_left)
                nc.vector.tensor_tensor(
                    colbuf[:].rearrange("p (j t) -> p t j", j=24)[:, 16 * h:16 * h + 16, :],
                    posall[:, hsl].rearrange("p (t j) -> p t j", j=24),
                    loc16b[:].rearrange("p (t j) -> p t j", j=24), ALU.add)
                for g2 in range(8):
                    src = colbuf[16 * g2:16 * (g2 + 1), :] \
                        .rearrange("p (j t) -> p j t", t=NT)[:, 0:KNN, 16 * h:16 * (h + 1)]
                    dst = wrapped[64 * h:64 * h + 16, :] \
                        .rearrange("p (j g t) -> p j g t", g=8, t=16)[:, :, g2, :]
                    nc.sync.dma_start(dst, src)
                for k in range(1, 4):
                    nc.sync.dma_start(
                        wrapped[64 * h + 16 * k:64 * h + 16 * (k + 1), :],
                        wrapped[64 * h:64 * h + 16, :])

            for t in range(NT):
                if t == 2:
                    emit_conv1()
                if t == 16:
                    emit_half_idx(0)
                lhs = aT[0:13, 128 * t:128 * (t + 1)]
                cand = selpool.tile([128, 128], F32, tag="cand")
                for h in range(2):
                    psd = pp.tile([128, HB], F32, tag="pp")
                    for j in range(4):
                        nc.tensor.matmul(
                            psd[:, 512 * j:512 * (j + 1)], lhs,
                            bT[0:13, HB * h + 512 * j:HB * h + 512 * (j + 1)])
                    dpk = dpool.tile([128, HB], U32, tag="dpk")
                    nc.vector.scalar_tensor_tensor(
                        dpk[:].rearrange("p (a c) -> p a c", c=CHUNK),
                        psd[:].bitcast(U32).rearrange("p (a c) -> p a c", c=CHUNK),
                        maskc[:, :],
                        iota512[:].rearrange("p (a c) -> p a c", a=1).broadcast_to([128, 8, CHUNK]),
                        ALU.bitwise_and, ALU.bitwise_or)
                    for c in range(8):
                        nc.vector.max(
                            cand[:, 64 * h + 8 * c:64 * h + 8 * (c + 1)],
                            dpk[:].bitcast(F32)[:, CHUNK * c:CHUNK * (c + 1)])
                v24 = v24all[:, 24 * t:24 * (t + 1)]
                pos = posall[:, 24 * t:24 * (t + 1)]
                c2 = selpool.tile([128, 128], F32, tag="c2")
                c3 = selpool.tile([128, 128], F32, tag="c3")
                nc.vector.max(v24[:, 0:8], cand[:])
                nc.vector.match_replace(c2[:], v24[:, 0:8], cand[:], NEG)
                nc.vector.max(v24[:, 8:16], c2[:])
                nc.vector.match_replace(c3[:], v24[:, 8:16], c2[:], NEG)
                nc.vector.max(v24[:, 16:24], c3[:])
                nc.vector.max_index(pos[:, 0:8], v24[:, 0:8], cand[:])
                nc.vector.max_index(pos[:, 8:16], v24[:, 8:16], cand[:])
                nc.vector.max_index(pos[:, 16:24], v24[:, 16:24], cand[:])

            emit_half_idx(1)

            ya3 = ya_dup[:].rearrange("p (m d) -> p m d", d=1)
            wri = wrapped[:].bitcast(I16)

            # ---- pass 1: streamed pair-gather -> h1 chunks -> bn stats ----
            M = gpool.tile([128, HB], F32, tag="M")
            nc.gpsimd.memset(M[:], NEG)
            h1sum = small("h1sum", (128, KNN))
            h1sq = small("h1sq", (128, KNN))
            sscr1 = chpool.tile([128, HB], F32, tag="sscr", bufs=1, name="sscr1")
            p1tiles = {}
            for qp in range(KNN // 2):
                gch = chpool.tile([128, 2 * HB], F32, tag="gch", bufs=2)
                p1tiles[qp] = gch
                nc.gpsimd.ap_gather(
                    gch[:], ya3, wri[:, 256 * qp:256 * (qp + 1)],
                    channels=128, num_elems=N, d=1, num_idxs=2 * HB)
                for jl in range(2):
                    q = 2 * qp + jl
                    g = gch[:, HB * jl:HB * (jl + 1)]
                    nc.vector.scalar_tensor_tensor(
                        g, g, 1.0, yb_p[:], ALU.mult, ALU.add,
                        accum_out=h1sum[:, q:q + 1])
                    nc.scalar.activation(
                        sscr1[:], g, AF.Square, accum_out=h1sq[:, q:q + 1])
                    if not has_conv2:
                        nc.vector.tensor_tensor(M[:], M[:], g, ALU.max)
            mv1 = mv_from_sums(h1sum, h1sq, NK)
            s1, b1 = scale_bias_from_mv(mv1)

            if has_conv2:
                # pass 2: normalize+lrelu, conv2 (block-diag), running max +
                # sums. The last two pass-1 pairs are still resident in the
                # gch ring with the yb add already applied, so process those
                # first without re-gathering; re-gather the rest.
                g1s = small("g1s", (128, KNN))
                ssq = small("h2sq", (128, KNN))
                sscr = chpool.tile([128, HB], F32, tag="sscr", bufs=1)
                NP = KNN // 2
                order = [NP - 2, NP - 1] + list(range(NP - 2))
                for qp in order:
                    if qp >= NP - 2:
                        gch = p1tiles[qp]
                    else:
                        gch = chpool.tile([128, 2 * HB], F32, tag="gch", bufs=2)
                        nc.gpsimd.ap_gather(
                            gch[:], ya3, wri[:, 256 * qp:256 * (qp + 1)],
                            channels=128, num_elems=N, d=1, num_idxs=2 * HB)
                        for jl in range(2):
                            g = gch[:, HB * jl:HB * (jl + 1)]
                            nc.vector.scalar_tensor_tensor(
                                g, g, 1.0, yb_p[:], ALU.mult, ALU.add)
                    grs = []
                    for jl in range(2):
                        q = 2 * qp + jl
                        g = gch[:, HB * jl:HB * (jl + 1)]
                        gr = chpool.tile([128, HB], MMDT, tag="gr", bufs=2)
                        nc.scalar.activation(
                            gr[:], g, AF.Prelu, bias=b1[:, :], scale=s1[:, :],
                            alpha=0.2, accum_out=g1s[:, q:q + 1])
                        grs.append(gr)
                    pscs = []
                    for jl in range(2):
                        psc = pp.tile([128, HB], F32, tag="pp")
                        if jl == 0:
                            # p-state warm-up: run two matmuls on the pre-norm
                            # gathered data into regions the real conv2 below
                            # overwrites (start=True resets the accumulation).
                            # They bridge the PE idle gap so the real burst
                            # runs at full clock.
                            for jj in range(2):
                                sl = slice(512 * jj, 512 * (jj + 1))
                                nc.tensor.matmul(psc[:, sl], wcD[bi][:],
                                                 gch[:, sl].bitcast(MMDT))
                        for jj in range(4):
                            sl = slice(512 * jj, 512 * (jj + 1))
                            nc.tensor.matmul(psc[:, sl], wcD[bi][:], grs[jl][:, sl])
                        pscs.append(psc)
                    for jl in range(2):
                        q = 2 * qp + jl
                        nc.vector.tensor_tensor(M[:], M[:], pscs[jl][:, 0:HB], ALU.max)
                        nc.scalar.activation(
                            sscr[:], pscs[jl][:, 0:HB], AF.Square, accum_out=ssq[:, q:q + 1])
                # sum(h2) per channel-half = W2 @ sum(g) (tiny matmuls)
                gsum = small("gsum", (128, 1))
                nc.vector.tensor_reduce(gsum[:], g1s[:], axis=AX.X, op=ALU.add)
                pss = pp.tile([128, HB], F32, tag="pp")
                nc.tensor.matmul(pss[0:64, 0:1], wcT[bi][0:64, :], gsum[0:64, 0:1])
                nc.tensor.matmul(pss[64:128, 0:1], wcT[bi][64:128, :], gsum[64:128, 0:1])
                ssum = small("h2sum", (128, 1))
                nc.scalar.copy(ssum[:], pss[:, 0:1])
                mv2 = mv_from_sums(ssum, ssq, NK)
                s2, b2 = scale_bias_from_mv(mv2)
                xout_p = x_p[bi + 1]
                nc.scalar.activation(
                    xout_p[:].rearrange("p (t g q) -> p g t q", t=16, g=8, q=16),
                    M[:], AF.Prelu, bias=b2[:, :], scale=s2[:, :], alpha=0.2)
            else:
                xout_p = x_p[bi + 1]
                nc.scalar.activation(
                    xout_p[:].rearrange("p (t g q) -> p g t q", t=16, g=8, q=16),
                    M[:], AF.Prelu, bias=b1[:, :], scale=s1[:, :], alpha=0.2)

            return xout_p

        xp = x_p[0]
        for bi in range(3):
            # head weights load after block 0 is issued so block-0 inputs
            # hit the DMA queues first
            if bi == 1:
                _head_loads()
            xp = edge_block(bi, xp, 4 if bi == 0 else 64)
        ectx.close()

        # ---------------- head ----------------
        w6T = HEADW["w6T"]; w7gT = HEADW["w7gT"]; w7xT = HEADW["w7xT"]
        w8T = HEADW["w8T"]; w9T = HEADW["w9T"]; id2 = HEADW["id2"]
        w6T12 = HEADW["w6T12"]; w7xT12 = HEADW["w7xT12"]
        x1_p, x2_p, x3_p = x_p[1], x_p[2], x_p[3]
        hb_pool = ctx.enter_context(tc.tile_pool(name="hb_pool", bufs=1))
        hu_pool = ctx.enter_context(tc.tile_pool(name="hu_pool", bufs=3))
        xb3 = hb_pool.tile([128, HB], BF16, tag="xb3")
        nc.vector.tensor_copy(xb3[:], x3_p[:].bitcast(F32))
        xb12 = []
        for h in range(2):
            t = hb_pool.tile([128, HB], BF16, tag=f"xb12_{h}")
            (nc.scalar.copy if h == 0 else nc.vector.tensor_copy)(
                t[0:64, :], x1_p[64 * h:64 * h + 64, :].bitcast(F32))
            (nc.scalar.copy if h == 1 else nc.vector.tensor_copy)(
                t[64:128, :], x2_p[64 * h:64 * h + 64, :].bitcast(F32))
            xb12.append(t)

        def head_stats_from(hs2, hq2):
            """hs2/hq2 [128,2] partial (sum, sumsq) halves -> s, b [128,1]."""
            mv = small("mvh", (128, 2))
            nc.vector.tensor_reduce(mv[:, 0:1], hs2[:], axis=AX.X, op=ALU.add)
            nc.vector.tensor_reduce(mv[:, 1:2], hq2[:], axis=AX.X, op=ALU.add)
            ts(mv[:, 0:1], mv[:, 0:1], 1.0 / N, ALU.mult)
            ts(mv[:, 1:2], mv[:, 1:2], 1.0 / N, ALU.mult)
            m2 = small("hm2")
            nc.vector.tensor_tensor(m2[:], mv[:, 0:1], mv[:, 0:1], ALU.mult)
            nc.vector.tensor_tensor(mv[:, 1:2], mv[:, 1:2], m2[:], ALU.subtract)
            ts(mv[:, 1:2], mv[:, 1:2], 1e-5, ALU.add)
            s = small("sh"); b = small("bh")
            rsqrt_inplace(s[:], mv[:, 1:2], 128)
            nc.vector.tensor_tensor(b[:], mv[:, 0:1], s[:], ALU.mult)
            ts(b[:], b[:], -1.0, ALU.mult)
            return s, b

        def head_sumsq(u, hq2, split=False):
            """Squares with accumulate: hq2[:, w] = sum(u[:, half w]^2).
            split=True puts one half on ACT to balance engine load."""
            uscr = hu_pool.tile([128, HB], F32, tag="uscr", bufs=2, name="uscr")
            for w in range(2):
                uh = u[:, HB * w:HB * (w + 1)]
                if split and w == 0:
                    nc.scalar.activation(
                        uscr[:], uh, AF.Square, accum_out=hq2[:, w:w + 1])
                else:
                    nc.vector.scalar_tensor_tensor(
                        uscr[:], uh, 1.0, uh, ALU.mult, ALU.mult,
                        accum_out=hq2[:, w:w + 1])

        gvecb = hb_pool.tile([128, 8], BF16, tag="gvecb")
        for g in range(8):
            u6 = hu_pool.tile([128, N], F32, tag="uh", name="u6")
            hs2 = small("hs", (128, 2)); hq2 = small("hq", (128, 2))
            for h in range(2):
                ps6 = pp.tile([128, HB], F32, tag="pp")
                for ci in range(4):
                    sl = slice(512 * ci, 512 * (ci + 1))
                    nc.tensor.matmul(
                        ps6[:, sl],
                        w6T12[:, 128 * g:128 * (g + 1)],
                        xb12[h][:, sl], start=True, stop=False)
                    nc.tensor.matmul(
                        ps6[:, sl],
                        w6T[2][64 * h:64 * h + 64, 128 * g:128 * (g + 1)],
                        xb3[64 * h:64 * h + 64, sl], start=False, stop=True)
                nc.scalar.activation(
                    u6[:, HB * h:HB * (h + 1)], ps6[:, 0:HB], AF.Identity,
                    accum_out=hs2[:, h:h + 1])
            # prelu is monotone increasing, so max_n prelu(s*u+b) =
            # prelu(s*max_n(u)+b): reduce the raw u6, then one scalar prelu.
            gmax = small("gmax")
            nc.vector.tensor_reduce(gmax[:], u6[:], axis=AX.X, op=ALU.max)
            head_sumsq(u6[:], hq2, split=True)
            s, b = head_stats_from(hs2, hq2)
            nc.scalar.activation(
                gvecb[:, g:g + 1], gmax[:], AF.Prelu, bias=b[:, :], scale=s[:, :],
                alpha=0.2)

        bias7 = hb_pool.tile([128, 4], F32, tag="bias7")
        ps7b = pp.tile([128, HB], F32, tag="pp")
        for og in range(4):
            for g in range(8):
                nc.tensor.matmul(
                    ps7b[:, og:og + 1],
                    w7gT[g][:, 128 * og:128 * (og + 1)],
                    gvecb[:, g:g + 1],
                    start=(g == 0), stop=(g == 7))
        nc.scalar.copy(bias7[:], ps7b[:, 0:4])

        h7b = []
        for og in range(4):
            u7 = hu_pool.tile([128, N], F32, tag="uh", name="u7")
            hs2 = small("hs", (128, 2)); hq2 = small("hq", (128, 2))
            for h in range(2):
                ps7 = pp.tile([128, HB], F32, tag="pp")
                for ci in range(4):
                    sl = slice(512 * ci, 512 * (ci + 1))
                    nc.tensor.matmul(
                        ps7[:, sl],
                        w7xT12[:, 128 * og:128 * (og + 1)],
                        xb12[h][:, sl], start=True, stop=False)
                    nc.tensor.matmul(
                        ps7[:, sl],
                        w7xT[2][64 * h:64 * h + 64, 128 * og:128 * (og + 1)],
                        xb3[64 * h:64 * h + 64, sl], start=False, stop=True)
                if h == 0:
                    nc.scalar.activation(
                        u7[:, HB * h:HB * (h + 1)], ps7[:, 0:HB],
                        AF.Identity, bias=bias7[:, og:og + 1],
                        accum_out=hs2[:, h:h + 1])
                else:
                    nc.vector.tensor_scalar(
                        u7[:, HB * h:HB * (h + 1)], ps7[:, 0:HB],
                        bias7[:, og:og + 1], None, ALU.add,
                        accum_out=hs2[:, h:h + 1])
            head_sumsq(u7[:], hq2)
            s, b = head_stats_from(hs2, hq2)
            t = hb_pool.tile([128, N], BF16, tag=f"h7b{og}")
            nc.scalar.activation(t[:], u7[:], AF.Prelu, bias=b[:, :], scale=s[:, :], alpha=0.2)
            h7b.append(t)

        h8b = []
        for og in range(2):
            u8 = hu_pool.tile([128, N], F32, tag="uh", name="u8")
            hs2 = small("hs", (128, 2)); hq2 = small("hq", (128, 2))
            for h in range(2):
                ps8 = pp.tile([128, HB], F32, tag="pp")
                for ci in range(4):
                    sl = slice(HB * h + 512 * ci, HB * h + 512 * (ci + 1))
                    psl = slice(512 * ci, 512 * (ci + 1))
                    for ki in range(4):
                        nc.tensor.matmul(
                            ps8[:, psl],
                            w8T[ki][:, 128 * og:128 * (og + 1)],
                            h7b[ki][:, sl],
                            start=(ki == 0), stop=(ki == 3))
                if h == 0:
                    nc.scalar.activation(
                        u8[:, HB * h:HB * (h + 1)], ps8[:, 0:HB], AF.Identity,
                        accum_out=hs2[:, h:h + 1])
                else:
                    nc.vector.tensor_scalar(
                        u8[:, HB * h:HB * (h + 1)], ps8[:, 0:HB],
                        1.0, None, ALU.mult, accum_out=hs2[:, h:h + 1])
            head_sumsq(u8[:], hq2)
            s, b = head_stats_from(hs2, hq2)
            t = hb_pool.tile([128, N], BF16, tag=f"h8b{og}")
            nc.scalar.activation(t[:], u8[:], AF.Prelu, bias=b[:, :], scale=s[:, :], alpha=0.2)
            h8b.append(t)

        o2 = hu_pool.tile([2, N], F32, tag="uh", name="o2")
        for ci in range(8):
            sl = slice(512 * ci, 512 * (ci + 1))
            ps9 = pp.tile([128, HB], F32, tag="pp")
            for ki in range(2):
                nc.tensor.matmul(
                    ps9[0:2, 0:512],
                    w9T[ki][:], h8b[ki][:, sl],
                    start=(ki == 0), stop=(ki == 1))
            nc.scalar.copy(o2[:, sl], ps9[0:2, 0:512])

        ost = hb_pool.tile([128, 64], F32, tag="ost")
        pst = pp.tile([128, HB], F32, tag="pp")
        for t in range(NT):
            nc.tensor.transpose(
                pst[:, 2 * t:2 * (t + 1)], o2[:, 128 * t:128 * (t + 1)], id2[:])
        nc.scalar.copy(ost[:], pst[:, 0:64])
        nc.sync.dma_start(
            out_d.rearrange("(t p) c -> p t c", p=128),
            ost[:].rearrange("p (t c) -> p t c", c=2))

    nc.finalize()
    return nc


def _shared_inputs(ws):
    import ml_dtypes
    w1, w2, w3, w4, w5, w6, w7, w8, w9 = ws
    f32 = np.float32
    bf16 = ml_dtypes.bfloat16
    d = {}
    for bi, w in [(0, w1), (1, w3), (2, w5)]:
        C = w.shape[1] // 2
        waT = np.ascontiguousarray(w[:, :C].T.astype(f32))          # [C, 64]
        wbT = np.ascontiguousarray((w[:, C:] - w[:, :C]).T.astype(f32))
        for h in range(2):
            blk = np.zeros((128, 128), dtype=f32)
            blk[64 * h:64 * h + C, 0:64] = waT
            blk[64 * h:64 * h + C, 64:128] = waT
            d[f"waD{bi}_h{h}"] = blk
        blk = np.zeros((128, 128), dtype=f32)
        blk[0:C, 0:64] = wbT
        blk[64:64 + C, 64:128] = wbT
        d[f"wbD{bi}"] = blk
    for bi, w in [(0, w2), (1, w4)]:
        wT = np.ascontiguousarray(w.T.astype(f32))                  # [64, 64]
        blk = np.zeros((128, 128), dtype=f32)
        blk[0:64, 0:64] = wT
        blk[64:128, 64:128] = wT
        d[f"wcD{bi}"] = blk
        d["w2T" if bi == 0 else "w4T"] = wT
    w6t = w6.T.astype(bf16); w7gt = w7[:, :1024].T.astype(bf16)
    w7xt = w7[:, 1024:].T.astype(bf16); w8t = w8.T.astype(bf16)
    w9t = w9.T.astype(bf16)
    for k in range(3):
        d[f"w6T_{k}"] = np.ascontiguousarray(w6t[64 * k:64 * (k + 1)])
        d[f"w7xT_{k}"] = np.ascontiguousarray(w7xt[64 * k:64 * (k + 1)])
    d["w6T12"] = np.ascontiguousarray(w6t[0:128])
    d["w7xT12"] = np.ascontiguousarray(w7xt[0:128])
    for k in range(8):
        d[f"w7gT_{k}"] = np.ascontiguousarray(w7gt[128 * k:128 * (k + 1)])
    for k in range(4):
        d[f"w8T_{k}"] = np.ascontiguousarray(w8t[128 * k:128 * (k + 1)])
    for k in range(2):
        d[f"w9T_{k}"] = np.ascontiguousarray(w9t[128 * k:128 * (k + 1)])
    d["iota512"] = np.broadcast_to(
        np.arange(CHUNK, dtype=np.uint32)[None, :], (128, CHUNK)).copy()
    d["maskc"] = np.full((128, 1), 0xFFFFFF00, dtype=np.uint32)
    d["magic"] = np.full((128, 1), 0x5F3759DF, dtype=np.uint32)
    d["id2"] = np.eye(2, dtype=f32)
    return d


def _run(inputs, want_debug=False):
    from concourse.bass_utils import run_bass_kernel_spmd

    if "nc" not in _CACHE:
        _CACHE["nc"] = _build_program()
    nc = _CACHE["nc"]

    x = np.asarray(inputs["x"], dtype=np.float32)
    ws = [np.asarray(inputs[f"w{i}"], dtype=np.float32) for i in range(1, 10)]
    shared = _shared_inputs(ws)
    in_maps = []
    for c in range(8):
        m = dict(shared)
        m["xt"] = np.ascontiguousarray(x[c % 4].T.astype(np.float32))
        in_maps.append(m)
    res = run_bass_kernel_spmd(nc, in_maps, list(range(8)))
    out = np.stack([res.results[c]["out"] for c in range(4)])
    if want_debug:
        return out, [res.results[c] for c in range(4)]
    return out


def kernel(**inputs):
    return _run(inputs)


# revision 20
# speedup vs baseline: 1.0116x; 1.0116x over previous
"""DGCNN semantic-segmentation kernel for 8x Trainium2 NeuronCores.

Strategy: data-parallel over batch. B=4 samples; core c processes sample c%4
end-to-end (cores 4-7 duplicate work so one SPMD program runs everywhere);
host takes outputs from cores 0-3. Everything for one sample stays on one
core: no cross-core communication.

Per-sample pipeline (all on device):
  3x EdgeConv blocks: kNN (fp16 hi/lo split K=13 distance matmul, ~1e-6
  accurate) -> top-20 selection (pack 8-bit chunk-local index into low
  mantissa bits, top-8 per 256-chunk via DVE max8, refine via
  max8/match_replace, recover columns arithmetically) -> streamed pair-gather
  via GPSIMD ap_gather (2 neighbors per instruction: gather cost is
  source-size bound, so pairs halve it) -> conv1 as packed block-diagonal
  float32r matmuls (1 cyc/row vs 4 for fp32) -> instance-norm stats via
  accum_out -> second conv as block-diagonal f32r matmul streamed with
  running max over k. Then the global head (w6..w9) with in1d norms, bf16
  matmuls, ACT-copy-with-accum for sums and DVE squares for sumsq.

Inter-block layout: packed [128, 2048] - partition p<64 holds channel p
for points n<2048, partition 64+p holds channel p for n>=2048.
"""

import sys

if "/opt/trn_rl_repo" not in sys.path:
    sys.path.insert(0, "/opt/trn_rl_repo")

import numpy as np

N = 4096
NT = 32          # row tiles of 128 for the distance/selection loop
HB = 2048
KNN = 20
NK = KNN * HB    # free size of the (virtual) packed h tensor per partition
CHUNK = 256
NEG = -3.0e38

# "all": conv1/conv2 matmuls in float32r (4x PE rate, ~2e-4 rounding);
# "none": same block-diagonal structure in plain fp32 (exact, 2x PE rate)
F32R_MODE = "none"

_CACHE = {}


def _build_program():
    import concourse.bacc as bacc
    import concourse.tile as tile
    from concourse import mybir
    from contextlib import ExitStack

    F32 = mybir.dt.float32
    F32R = mybir.dt.float32r
    F16 = mybir.dt.float16
    BF16 = mybir.dt.bfloat16
    U32 = mybir.dt.uint32
    U16 = mybir.dt.uint16
    I16 = mybir.dt.int16
    AF = mybir.ActivationFunctionType
    ALU = mybir.AluOpType
    AX = mybir.AxisListType

    nc = bacc.Bacc("TRN2", target_bir_lowering=False, debug=False, num_devices=8)

    def din(name, shape, dt=F32):
        return nc.dram_tensor(name, shape, dt, kind="ExternalInput").ap()

    xt_d = din("xt", [4, N])
    # block-diagonal conv1/conv2 weights, one [128,128] block per use
    waD_d = [[din(f"waD{i}_h{h}", [128, 128]) for h in range(2)] for i in range(3)]
    wbD_d = [din(f"wbD{i}", [128, 128]) for i in range(3)]
    wcD_d = [din(f"wcD{i}", [128, 128]) for i in range(2)]
    wcT_d = [din("w2T", [64, 64]), din("w4T", [64, 64])]
    w6T_d = [din(f"w6T_{k}", [64, 1024], BF16) for k in range(3)]
    w6T12_d = din("w6T12", [128, 1024], BF16)
    w7xT12_d = din("w7xT12", [128, 512], BF16)
    w7gT_d = [din(f"w7gT_{k}", [128, 512], BF16) for k in range(8)]
    w7xT_d = [din(f"w7xT_{k}", [64, 512], BF16) for k in range(3)]
    w8T_d = [din(f"w8T_{k}", [128, 256], BF16) for k in range(4)]
    w9T_d = [din(f"w9T_{k}", [128, 2], BF16) for k in range(2)]
    iota512_d = din("iota512", [128, CHUNK], U32)
    maskc_d = din("maskc", [128, 1], U32)
    magic_d = din("magic", [128, 1], U32)
    id2_d = din("id2", [2, 2])

    out_d = nc.dram_tensor("out", [N, 2], F32, kind="ExternalOutput").ap()

    with tile.TileContext(nc) as tc, ExitStack() as ctx:
        wpool = ctx.enter_context(tc.tile_pool(name="wpool", bufs=1))
        xpool = ctx.enter_context(tc.tile_pool(name="xpool", bufs=1))
        stpool = ctx.enter_context(tc.tile_pool(name="stpool", bufs=1))
        pp = ctx.enter_context(tc.tile_pool(name="pp", bufs=2, space="PSUM"))
        ectx = ExitStack()
        abpool = ectx.enter_context(tc.tile_pool(name="abpool", bufs=1))
        gpool = ectx.enter_context(tc.tile_pool(name="gpool", bufs=1))
        dpool = ectx.enter_context(tc.tile_pool(name="dpool", bufs=2))
        selpool = ectx.enter_context(tc.tile_pool(name="selpool", bufs=4))
        idxpool = ectx.enter_context(tc.tile_pool(name="idxpool", bufs=1))
        chpool = ectx.enter_context(tc.tile_pool(name="chpool", bufs=1))

        def load(pool, ap_d, shape, dt=F32, dup64=False):
            rows = shape[0]
            tshape = [128, shape[1]] if dup64 else shape
            t = pool.tile(tshape, dt, tag=ap_d.tensor.name, name=ap_d.tensor.name + "_sb")
            nc.sync.dma_start(t[0:rows, :], ap_d)
            if dup64:
                nc.sync.dma_start(t[64:64 + rows, :], ap_d)
            return t

        MMDT = F32R if F32R_MODE == "all" else F32

        def load_f32r(ap_d):
            """DMA an f32 [128,128] block weight; round to f32r via DVE copy."""
            if MMDT is F32:
                t = wpool.tile([128, 128], F32, tag=ap_d.tensor.name,
                               name=ap_d.tensor.name + "_sb")
                nc.sync.dma_start(t[:], ap_d)
                return t
            st = wpool.tile([128, 128], F32, tag="wstage", bufs=2,
                            name=ap_d.tensor.name + "_st")
            nc.sync.dma_start(st[:], ap_d)
            t = wpool.tile([128, 128], F32R, tag=ap_d.tensor.name,
                           name=ap_d.tensor.name + "_sb")
            nc.vector.tensor_copy(t[:], st[:])
            return t

        # ---- early loads: input + block-structure constants ----
        xt_sb = gpool.tile([4, N], F32, tag="ya_dup", name="xt_sb")
        nc.sync.dma_start(xt_sb[:], xt_d)
        iota512 = load(wpool, iota512_d, [128, CHUNK], U32)
        maskc = load(wpool, maskc_d, [128, 1], U32)
        magic = load(wpool, magic_d, [128, 1], U32)
        waD = [[load_f32r(waD_d[0][h]) for h in range(2)], None, None]
        wbD = [load_f32r(wbD_d[0]), None, None]
        wcD = [load_f32r(wcD_d[0]), None]
        wcT = [load(wpool, wcT_d[0], [64, 64], dup64=True),
               load(wpool, wcT_d[1], [64, 64], dup64=True)]

        x_p = [xpool.tile([128, HB], MMDT, tag=f"x{i}_p", name=f"x{i}_p") for i in range(4)]
        # zero-fill first: block-diagonal matmuls stream all 128 partitions, so
        # unused rows must be finite (0 * w = 0, but NaN * 0 = NaN)
        nc.gpsimd.memset(x_p[0][:].bitcast(F32), 0.0)
        nc.vector.tensor_copy(x_p[0][0:4, :], xt_sb[:, 0:HB])
        nc.vector.tensor_copy(x_p[0][64:68, :], xt_sb[:, HB:N])

        HEADW = {}

        def _head_loads():
            for i in (1, 2):
                waD[i] = [load_f32r(waD_d[i][h]) for h in range(2)]
                wbD[i] = load_f32r(wbD_d[i])
            wcD[1] = load_f32r(wcD_d[1])
            HEADW["w6T"] = [None, None, load(wpool, w6T_d[2], [64, 1024], BF16, dup64=True)]
            HEADW["w6T12"] = load(wpool, w6T12_d, [128, 1024], BF16)
            HEADW["w7xT12"] = load(wpool, w7xT12_d, [128, 512], BF16)
            HEADW["w7gT"] = [load(wpool, a, [128, 512], BF16) for a in w7gT_d]
            HEADW["w7xT"] = [None, None, load(wpool, w7xT_d[2], [64, 512], BF16, dup64=True)]
            HEADW["w8T"] = [load(wpool, a, [128, 256], BF16) for a in w8T_d]
            HEADW["w9T"] = [load(wpool, a, [128, 2], BF16) for a in w9T_d]
            HEADW["id2"] = load(wpool, id2_d, [2, 2])

        def small(tag, shape=(128, 1), dt=F32):
            return stpool.tile(list(shape), dt, tag=tag, name=tag)

        def ts(out, in0, s1, op0, s2=None, op1=None):
            if op1 is None:
                nc.vector.tensor_scalar(out, in0, s1, None, op0)
            else:
                nc.vector.tensor_scalar(out, in0, s1, s2, op0, op1)

        def rsqrt_inplace(y, t_in, rows):
            b = small("rs_b", (rows, 1), U32)
            ts(b[:], t_in.bitcast(U32), 1, ALU.logical_shift_right)
            nc.vector.tensor_tensor(y.bitcast(U32), magic[0:rows, :], b[:], ALU.subtract)
            for _ in range(2):
                u = small("rs_u", (rows, 1))
                nc.vector.tensor_tensor(u[:], y, y, ALU.mult)
                nc.vector.tensor_tensor(u[:], u[:], t_in, ALU.mult)
                ts(u[:], u[:], -0.5, ALU.mult, 1.5, ALU.add)
                nc.vector.tensor_tensor(y, y, u[:], ALU.mult)

        def scale_bias_from_mv(mv):
            """mv [128,2] per-partition (mean, var); rows p/p+64 are halves of one
            channel. Returns s128, b128 [128,1] with normalized = s*x + b."""
            mvb = small("st_mvb", (64, 2))
            nc.sync.dma_start(mvb[:], mv[64:128, :])
            m = small("st_m", (64, 1)); v = small("st_v", (64, 1))
            dm = small("st_dm", (64, 1))
            nc.vector.tensor_tensor(m[:], mv[0:64, 0:1], mvb[:, 0:1], ALU.add)
            ts(m[:], m[:], 0.5, ALU.mult)
            nc.vector.tensor_tensor(v[:], mv[0:64, 1:2], mvb[:, 1:2], ALU.add)
            nc.vector.tensor_tensor(dm[:], mv[0:64, 0:1], mvb[:, 0:1], ALU.subtract)
            nc.vector.tensor_tensor(dm[:], dm[:], dm[:], ALU.mult)
            ts(v[:], v[:], 0.5, ALU.mult)
            ts(dm[:], dm[:], 0.25, ALU.mult)
            nc.vector.tensor_tensor(v[:], v[:], dm[:], ALU.add)
            ts(v[:], v[:], 1e-5, ALU.add)
            s = small("st_s", (64, 1))
            rsqrt_inplace(s[:], v[:], 64)
            bb = small("st_bb", (64, 1))
            nc.vector.tensor_tensor(bb[:], m[:], s[:], ALU.mult)
            ts(bb[:], bb[:], -1.0, ALU.mult)
            s128 = small("st_s128"); b128 = small("st_b128")
            nc.vector.tensor_copy(s128[0:64, :], s[:])
            nc.vector.tensor_copy(b128[0:64, :], bb[:])
            nc.sync.dma_start(s128[64:128, :], s[:])
            nc.sync.dma_start(b128[64:128, :], bb[:])
            return s128, b128

        def mv_from_sums(ssum, ssq, count):
            """[128, w] partial sums -> mv [128, 2] (mean, var per partition)."""
            mv = small("sm_mv", (128, 2))
            nc.vector.tensor_reduce(mv[:, 0:1], ssum[:], axis=AX.X, op=ALU.add)
            nc.vector.tensor_reduce(mv[:, 1:2], ssq[:], axis=AX.X, op=ALU.add)
            ts(mv[:, 0:1], mv[:, 0:1], 1.0 / count, ALU.mult)
            ts(mv[:, 1:2], mv[:, 1:2], 1.0 / count, ALU.mult)
            m2 = small("sm_m2")
            nc.vector.tensor_tensor(m2[:], mv[:, 0:1], mv[:, 0:1], ALU.mult)
            nc.vector.tensor_tensor(mv[:, 1:2], mv[:, 1:2], m2[:], ALU.subtract)
            return mv

        # ---------------- EdgeConv block ----------------
        def edge_block(bi, xin_p, C):
            has_conv2 = bi < 2
            xin_f = xin_p[:].bitcast(F32)

            # distance operand prep: pieces computed at partition base 0 (ACT
            # alignment rule), assembled into aT/bT via DMAs.
            sq = dpool.tile([128, HB], F32, tag="dpk", name="sq")
            nc.scalar.activation(sq[0:3, :], xin_f[0:3, :], AF.Square)
            nc.scalar.activation(sq[64:67, :], xin_f[64:67, :], AF.Square)
            ones3 = abpool.tile([128, 1], F32, tag="ones3")
            nc.gpsimd.memset(ones3[:], 1.0)

            src4 = gpool.tile([4, N], F32, tag="ya_dup", name="src4")
            for h in range(2):
                psx = pp.tile([128, HB], F32, tag="pp")
                if h == 0:
                    # p-state warm-up; overwritten by the real psx below
                    for j in range(2):
                        nc.tensor.matmul(
                            psx[0:64, 512 * j:512 * (j + 1)],
                            wcT[0][0:64, :],
                            sq[0:64, 512 * j:512 * (j + 1)])
                for j in range(4):
                    nc.tensor.matmul(
                        psx[0:1, 512 * j:512 * (j + 1)],
                        ones3[64 * h:64 * h + 3, 0:1],
                        sq[64 * h:64 * h + 3, 512 * j:512 * (j + 1)])
                nc.scalar.copy(src4[0:1, HB * h:HB * (h + 1)], psx[0:1, 0:HB])
                nc.sync.dma_start(src4[1:4, HB * h:HB * (h + 1)], xin_f[64 * h:64 * h + 3, :])

            hi4 = gpool.tile([4, N], F16, tag="M", name="hi4")
            lo4 = gpool.tile([4, N], F16, tag="yb_p", name="lo4")
            nc.scalar.copy(hi4[:], src4[:])
            nc.vector.scalar_tensor_tensor(
                lo4[:], hi4[:], -1.0, src4[:], ALU.mult, ALU.add)
            nh4 = dpool.tile([4, N], F16, tag="dpk", name="nh4")
            nl4 = dpool.tile([4, N], F16, tag="dpk", name="nl4")
            nc.scalar.mul(nh4[:], hi4[:], -1.0)
            nc.scalar.mul(nl4[:], lo4[:], -1.0)

            # aT rows: [1, 1, -xxh, -xxl, 2ph(3), 2ph(3), 2pl(3)]
            # bT rows: [-xxh, -xxl, 1, 1, ph(3), pl(3), ph(3)]
            aT = abpool.tile([16, N], F16, tag="aT")
            bT = abpool.tile([16, N], F16, tag="bT")
            nc.gpsimd.memset(aT[0:2, :], 1.0)
            nc.sync.dma_start(bT[2:4, :], aT[0:2, :])
            nc.sync.dma_start(aT[2:3, :], nh4[0:1, :])
            nc.sync.dma_start(aT[3:4, :], nl4[0:1, :])
            nc.sync.dma_start(bT[0:1, :], nh4[0:1, :])
            nc.sync.dma_start(bT[1:2, :], nl4[0:1, :])
            h2x = dpool.tile([4, N], F16, tag="dpk", name="h2x")
            l2x = dpool.tile([4, N], F16, tag="dpk", name="l2x")
            ts(h2x[:], hi4[:], 2.0, ALU.mult)
            ts(l2x[:], lo4[:], 2.0, ALU.mult)
            nc.sync.dma_start(aT[4:7, :], h2x[1:4, :])
            nc.sync.dma_start(aT[7:10, :], h2x[1:4, :])
            nc.sync.dma_start(aT[10:13, :], l2x[1:4, :])
            nc.sync.dma_start(bT[4:7, :], hi4[1:4, :])
            nc.sync.dma_start(bT[7:10, :], lo4[1:4, :])
            nc.sync.dma_start(bT[10:13, :], hi4[1:4, :])

            # ya (duplicated to both partition halves, full n) and yb (packed)
            # via packed block-diagonal matmuls. Issued from inside the
            # selection loop: they are only needed by the gather passes, and
            # issuing them here keeps them off the pre-selection critical path
            # (PE has slack while the DVE runs selection).
            ya_dup = gpool.tile([128, N], F32, tag="ya_dup")
            yb_p = gpool.tile([128, HB], F32, tag="yb_p")

            def emit_conv1():
                for h in range(2):
                    psy = pp.tile([128, HB], F32, tag="pp")
                    for j in range(4):
                        sl = slice(512 * j, 512 * (j + 1))
                        nc.tensor.matmul(psy[:, sl], waD[bi][h][:], xin_p[:, sl])
                    nc.scalar.copy(ya_dup[:, HB * h:HB * (h + 1)], psy[:, 0:HB])
                psb = pp.tile([128, HB], F32, tag="pp")
                for j in range(4):
                    sl = slice(512 * j, 512 * (j + 1))
                    nc.tensor.matmul(psb[:, sl], wbD[bi][:], xin_p[:, sl])
                nc.scalar.copy(
                    yb_p[:].rearrange("p (g t q) -> p t g q", g=8, t=16, q=16),
                    psb[:, 0:HB].rearrange("p (t g q) -> p t g q", t=16, g=8, q=16))

            # ---- distance + selection ----
            colbuf = idxpool.tile([128, 768], U16, tag="colbuf")
            posall = idxpool.tile([128, 768], U16, tag="posall")
            v24all = idxpool.tile([128, 768], F32, tag="v24all")
            # wrapped gather index lists (per half): position i = j*2048+n_loc;
            # stored wrapped-16: partition 16k + (i%16), free i//16
            wrapped = idxpool.tile([128, 2560], U16, tag="wrapped")

            def emit_half_idx(h):
                # column arithmetic for half h's tiles, then reformat DMAs.
                # Issued right after the half's last selection tile so the DMA
                # chain hides under the other half's selection.
                hsl = slice(384 * h, 384 * (h + 1))
                locb = idxpool.tile([128, 384], U32, tag="locb")
                ts(locb[:], v24all[:, hsl].bitcast(U32), 255, ALU.bitwise_and)
                loc16b = idxpool.tile([128, 384], U16, tag="loc16b")
                nc.vector.tensor_copy(loc16b[:], locb[:])
                ts(posall[:, hsl], posall[:, hsl], 3, ALU.logical_shift_right)
                ts(posall[:, hsl], posall[:, hsl], 8, ALU.logical_shift_left)
                nc.vector.tensor_tensor(
                    colbuf[:].rearrange("p (j t) -> p t j", j=24)[:, 16 * h:16 * h + 16, :],
                    posall[:, hsl].rearrange("p (t j) -> p t j", j=24),
                    loc16b[:].rearrange("p (t j) -> p t j", j=24), ALU.add)
                for g2 in range(8):
                    src = colbuf[16 * g2:16 * (g2 + 1), :] \
                        .rearrange("p (j t) -> p j t", t=NT)[:, 0:KNN, 16 * h:16 * (h + 1)]
                    dst = wrapped[64 * h:64 * h + 16, :] \
                        .rearrange("p (j g t) -> p j g t", g=8, t=16)[:, :, g2, :]
                    nc.sync.dma_start(dst, src)
                for k in range(1, 4):
                    nc.sync.dma_start(
                        wrapped[64 * h + 16 * k:64 * h + 16 * (k + 1), :],
                        wrapped[64 * h:64 * h + 16, :])

            for t in range(NT):
                if t == 2:
                    emit_conv1()
                if t == 16:
                    emit_half_idx(0)
                lhs = aT[0:13, 128 * t:128 * (t + 1)]
                cand = selpool.tile([128, 128], F32, tag="cand")
                for h in range(2):
                    psd = pp.tile([128, HB], F32, tag="pp")
                    for j in range(4):
                        nc.tensor.matmul(
                            psd[:, 512 * j:512 * (j + 1)], lhs,
                            bT[0:13, HB * h + 512 * j:HB * h + 512 * (j + 1)])
                    dpk = dpool.tile([128, HB], U32, tag="dpk")
                    nc.vector.scalar_tensor_tensor(
                        dpk[:].rearrange("p (a c) -> p a c", c=CHUNK),
                        psd[:].bitcast(U32).rearrange("p (a c) -> p a c", c=CHUNK),
                        maskc[:, :],
                        iota512[:].rearrange("p (a c) -> p a c", a=1).broadcast_to([128, 8, CHUNK]),
                        ALU.bitwise_and, ALU.bitwise_or)
                    for c in range(8):
                        nc.vector.max(
                            cand[:, 64 * h + 8 * c:64 * h + 8 * (c + 1)],
                            dpk[:].bitcast(F32)[:, CHUNK * c:CHUNK * (c + 1)])
                v24 = v24all[:, 24 * t:24 * (t + 1)]
                pos = posall[:, 24 * t:24 * (t + 1)]
                c2 = selpool.tile([128, 128], F32, tag="c2")
                c3 = selpool.tile([128, 128], F32, tag="c3")
                nc.vector.max(v24[:, 0:8], cand[:])
                nc.vector.match_replace(c2[:], v24[:, 0:8], cand[:], NEG)
                nc.vector.max(v24[:, 8:16], c2[:])
                nc.vector.match_replace(c3[:], v24[:, 8:16], c2[:], NEG)
                nc.vector.max(v24[:, 16:24], c3[:])
                nc.vector.max_index(pos[:, 0:8], v24[:, 0:8], cand[:])
                nc.vector.max_index(pos[:, 8:16], v24[:, 8:16], cand[:])
                nc.vector.max_index(pos[:, 16:24], v24[:, 16:24], cand[:])

            emit_half_idx(1)

            ya3 = ya_dup[:].rearrange("p (m d) -> p m d", d=1)
            wri = wrapped[:].bitcast(I16)

            # ---- pass 1: streamed pair-gather -> h1 chunks -> bn stats ----
            M = gpool.tile([128, HB], F32, tag="M")
            Mb = gpool.tile([128, HB], BF16, tag="Mb")
            if has_conv2:
                nc.gpsimd.memset(M[:], NEG)
            else:
                nc.gpsimd.memset(Mb[:], NEG)
            h1sum = small("h1sum", (128, KNN))
            h1sq = small("h1sq", (128, KNN))
            sscr1 = chpool.tile([128, HB], F32, tag="sscr", bufs=1, name="sscr1")
            p1tiles = {}
            for qp in range(KNN // 2):
                gch = chpool.tile([128, 2 * HB], F32, tag="gch", bufs=2)
                p1tiles[qp] = gch
                nc.gpsimd.ap_gather(
                    gch[:], ya3, wri[:, 256 * qp:256 * (qp + 1)],
                    channels=128, num_elems=N, d=1, num_idxs=2 * HB)
                for jl in range(2):
                    q = 2 * qp + jl
                    g = gch[:, HB * jl:HB * (jl + 1)]
                    if has_conv2:
                        nc.vector.scalar_tensor_tensor(
                            g, g, 1.0, yb_p[:], ALU.mult, ALU.add,
                            accum_out=h1sum[:, q:q + 1])
                        nc.scalar.activation(
                            sscr1[:], g, AF.Square, accum_out=h1sq[:, q:q + 1])
                    else:
                        # x3 feeds only the head, so the running max can run
                        # in bf16 (2x DVE tensor_tensor). Stats stay fp32 via
                        # the accumulators.
                        gb = chpool.tile([128, HB], BF16, tag="gb", bufs=2)
                        nc.vector.scalar_tensor_tensor(
                            gb[:], g, 1.0, yb_p[:], ALU.mult, ALU.add,
                            accum_out=h1sum[:, q:q + 1])
                        nc.scalar.activation(
                            sscr1[:], g, AF.Square, accum_out=h1sq[:, q:q + 1])
                        nc.vector.tensor_tensor(Mb[:], Mb[:], gb[:], ALU.max)
            mv1 = mv_from_sums(h1sum, h1sq, NK)
            s1, b1 = scale_bias_from_mv(mv1)

            if has_conv2:
                # pass 2: normalize+lrelu, conv2 (block-diag), running max +
                # sums. The last two pass-1 pairs are still resident in the
                # gch ring with the yb add already applied, so process those
                # first without re-gathering; re-gather the rest.
                g1s = small("g1s", (128, KNN))
                ssq = small("h2sq", (128, KNN))
                sscr = chpool.tile([128, HB], F32, tag="sscr", bufs=1)
                NP = KNN // 2
                order = [NP - 2, NP - 1] + list(range(NP - 2))
                for qp in order:
                    if qp >= NP - 2:
                        gch = p1tiles[qp]
                    else:
                        gch = chpool.tile([128, 2 * HB], F32, tag="gch", bufs=2)
                        nc.gpsimd.ap_gather(
                            gch[:], ya3, wri[:, 256 * qp:256 * (qp + 1)],
                            channels=128, num_elems=N, d=1, num_idxs=2 * HB)
                        for jl in range(2):
                            g = gch[:, HB * jl:HB * (jl + 1)]
                            nc.vector.scalar_tensor_tensor(
                                g, g, 1.0, yb_p[:], ALU.mult, ALU.add)
                    grs = []
                    for jl in range(2):
                        q = 2 * qp + jl
                        g = gch[:, HB * jl:HB * (jl + 1)]
                        gr = chpool.tile([128, HB], MMDT, tag="gr", bufs=2)
                        nc.scalar.activation(
                            gr[:], g, AF.Prelu, bias=b1[:, :], scale=s1[:, :],
                            alpha=0.2, accum_out=g1s[:, q:q + 1])
                        grs.append(gr)
                    pscs = []
                    for jl in range(2):
                        psc = pp.tile([128, HB], F32, tag="pp")
                        if jl == 0:
                            # p-state warm-up: run two matmuls on the pre-norm
                            # gathered data into regions the real conv2 below
                            # overwrites (start=True resets the accumulation).
                            # They bridge the PE idle gap so the real burst
                            # runs at full clock.
                            for jj in range(2):
                                sl = slice(512 * jj, 512 * (jj + 1))
                                nc.tensor.matmul(psc[:, sl], wcD[bi][:],
                                                 gch[:, sl].bitcast(MMDT))
                        for jj in range(4):
                            sl = slice(512 * jj, 512 * (jj + 1))
                            nc.tensor.matmul(psc[:, sl], wcD[bi][:], grs[jl][:, sl])
                        pscs.append(psc)
                    for jl in range(2):
                        q = 2 * qp + jl
                        nc.vector.tensor_tensor(M[:], M[:], pscs[jl][:, 0:HB], ALU.max)
                        nc.scalar.activation(
                            sscr[:], pscs[jl][:, 0:HB], AF.Square, accum_out=ssq[:, q:q + 1])
                # sum(h2) per channel-half = W2 @ sum(g) (tiny matmuls)
                gsum = small("gsum", (128, 1))
                nc.vector.tensor_reduce(gsum[:], g1s[:], axis=AX.X, op=ALU.add)
                pss = pp.tile([128, HB], F32, tag="pp")
                nc.tensor.matmul(pss[0:64, 0:1], wcT[bi][0:64, :], gsum[0:64, 0:1])
                nc.tensor.matmul(pss[64:128, 0:1], wcT[bi][64:128, :], gsum[64:128, 0:1])
                ssum = small("h2sum", (128, 1))
                nc.scalar.copy(ssum[:], pss[:, 0:1])
                mv2 = mv_from_sums(ssum, ssq, NK)
                s2, b2 = scale_bias_from_mv(mv2)
                xout_p = x_p[bi + 1]
                nc.scalar.activation(
                    xout_p[:].rearrange("p (t g q) -> p g t q", t=16, g=8, q=16),
                    M[:], AF.Prelu, bias=b2[:, :], scale=s2[:, :], alpha=0.2)
            else:
                xout_p = x_p[bi + 1]
                nc.scalar.activation(
                    xout_p[:].rearrange("p (t g q) -> p g t q", t=16, g=8, q=16),
                    Mb[:], AF.Prelu, bias=b1[:, :], scale=s1[:, :], alpha=0.2)

            return xout_p

        xp = x_p[0]
        for bi in range(3):
            # head weights load after block 0 is issued so block-0 inputs
            # hit the DMA queues first
            if bi == 1:
                _head_loads()
            xp = edge_block(bi, xp, 4 if bi == 0 else 64)
        ectx.close()

        # ---------------- head ----------------
        w6T = HEADW["w6T"]; w7gT = HEADW["w7gT"]; w7xT = HEADW["w7xT"]
        w8T = HEADW["w8T"]; w9T = HEADW["w9T"]; id2 = HEADW["id2"]
        w6T12 = HEADW["w6T12"]; w7xT12 = HEADW["w7xT12"]
        x1_p, x2_p, x3_p = x_p[1], x_p[2], x_p[3]
        hb_pool = ctx.enter_context(tc.tile_pool(name="hb_pool", bufs=1))
        hu_pool = ctx.enter_context(tc.tile_pool(name="hu_pool", bufs=3))
        xb3 = hb_pool.tile([128, HB], BF16, tag="xb3")
        nc.vector.tensor_copy(xb3[:], x3_p[:].bitcast(F32))
        xb12 = []
        for h in range(2):
            t = hb_pool.tile([128, HB], BF16, tag=f"xb12_{h}")
            (nc.scalar.copy if h == 0 else nc.vector.tensor_copy)(
                t[0:64, :], x1_p[64 * h:64 * h + 64, :].bitcast(F32))
            (nc.scalar.copy if h == 1 else nc.vector.tensor_copy)(
                t[64:128, :], x2_p[64 * h:64 * h + 64, :].bitcast(F32))
            xb12.append(t)

        def head_stats_from(hs2, hq2):
            """hs2/hq2 [128,2] partial (sum, sumsq) halves -> s, b [128,1]."""
            mv = small("mvh", (128, 2))
            nc.vector.tensor_reduce(mv[:, 0:1], hs2[:], axis=AX.X, op=ALU.add)
            nc.vector.tensor_reduce(mv[:, 1:2], hq2[:], axis=AX.X, op=ALU.add)
            ts(mv[:, 0:1], mv[:, 0:1], 1.0 / N, ALU.mult)
            ts(mv[:, 1:2], mv[:, 1:2], 1.0 / N, ALU.mult)
            m2 = small("hm2")
            nc.vector.tensor_tensor(m2[:], mv[:, 0:1], mv[:, 0:1], ALU.mult)
            nc.vector.tensor_tensor(mv[:, 1:2], mv[:, 1:2], m2[:], ALU.subtract)
            ts(mv[:, 1:2], mv[:, 1:2], 1e-5, ALU.add)
            s = small("sh"); b = small("bh")
            rsqrt_inplace(s[:], mv[:, 1:2], 128)
            nc.vector.tensor_tensor(b[:], mv[:, 0:1], s[:], ALU.mult)
            ts(b[:], b[:], -1.0, ALU.mult)
            return s, b

        def head_sumsq(u, hq2, split=False):
            """Squares with accumulate: hq2[:, w] = sum(u[:, half w]^2).
            split=True puts one half on ACT to balance engine load."""
            uscr = hu_pool.tile([128, HB], F32, tag="uscr", bufs=2, name="uscr")
            for w in range(2):
                uh = u[:, HB * w:HB * (w + 1)]
                if split and w == 0:
                    nc.scalar.activation(
                        uscr[:], uh, AF.Square, accum_out=hq2[:, w:w + 1])
                else:
                    nc.vector.scalar_tensor_tensor(
                        uscr[:], uh, 1.0, uh, ALU.mult, ALU.mult,
                        accum_out=hq2[:, w:w + 1])

        gvecb = hb_pool.tile([128, 8], BF16, tag="gvecb")
        for g in range(8):
            u6 = hu_pool.tile([128, N], F32, tag="uh", name="u6")
            hs2 = small("hs", (128, 2)); hq2 = small("hq", (128, 2))
            for h in range(2):
                ps6 = pp.tile([128, HB], F32, tag="pp")
                for ci in range(4):
                    sl = slice(512 * ci, 512 * (ci + 1))
                    nc.tensor.matmul(
                        ps6[:, sl],
                        w6T12[:, 128 * g:128 * (g + 1)],
                        xb12[h][:, sl], start=True, stop=False)
                    nc.tensor.matmul(
                        ps6[:, sl],
                        w6T[2][64 * h:64 * h + 64, 128 * g:128 * (g + 1)],
                        xb3[64 * h:64 * h + 64, sl], start=False, stop=True)
                nc.scalar.activation(
                    u6[:, HB * h:HB * (h + 1)], ps6[:, 0:HB], AF.Identity,
                    accum_out=hs2[:, h:h + 1])
            # prelu is monotone increasing, so max_n prelu(s*u+b) =
            # prelu(s*max_n(u)+b): reduce the raw u6, then one scalar prelu.
            gmax = small("gmax")
            nc.vector.tensor_reduce(gmax[:], u6[:], axis=AX.X, op=ALU.max)
            head_sumsq(u6[:], hq2, split=True)
            s, b = head_stats_from(hs2, hq2)
            nc.scalar.activation(
                gvecb[:, g:g + 1], gmax[:], AF.Prelu, bias=b[:, :], scale=s[:, :],
                alpha=0.2)

        bias7 = hb_pool.tile([128, 4], F32, tag="bias7")
        ps7b = pp.tile([128, HB], F32, tag="pp")
        for og in range(4):
            for g in range(8):
                nc.tensor.matmul(
                    ps7b[:, og:og + 1],
                    w7gT[g][:, 128 * og:128 * (og + 1)],
                    gvecb[:, g:g + 1],
                    start=(g == 0), stop=(g == 7))
        nc.scalar.copy(bias7[:], ps7b[:, 0:4])

        h7b = []
        for og in range(4):
            u7 = hu_pool.tile([128, N], F32, tag="uh", name="u7")
            hs2 = small("hs", (128, 2)); hq2 = small("hq", (128, 2))
            for h in range(2):
                ps7 = pp.tile([128, HB], F32, tag="pp")
                for ci in range(4):
                    sl = slice(512 * ci, 512 * (ci + 1))
                    nc.tensor.matmul(
                        ps7[:, sl],
                        w7xT12[:, 128 * og:128 * (og + 1)],
                        xb12[h][:, sl], start=True, stop=False)
                    nc.tensor.matmul(
                        ps7[:, sl],
                        w7xT[2][64 * h:64 * h + 64, 128 * og:128 * (og + 1)],
                        xb3[64 * h:64 * h + 64, sl], start=False, stop=True)
                nc.scalar.activation(
                    u7[:, HB * h:HB * (h + 1)], ps7[:, 0:HB],
                    AF.Identity, bias=bias7[:, og:og + 1],
                    accum_out=hs2[:, h:h + 1])
            head_sumsq(u7[:], hq2)
            s, b = head_stats_from(hs2, hq2)
            t = hb_pool.tile([128, N], BF16, tag=f"h7b{og}")
            nc.scalar.activation(t[:], u7[:], AF.Prelu, bias=b[:, :], scale=s[:, :], alpha=0.2)
            h7b.append(t)

        h8b = []
        for og in range(2):
            u8 = hu_pool.tile([128, N], F32, tag="uh", name="u8")
            hs2 = small("hs", (128, 2)); hq2 = small("hq", (128, 2))
            for h in range(2):
                ps8 = pp.tile([128, HB], F32, tag="pp")
                for ci in range(4):
                    sl = slice(HB * h + 512 * ci, HB * h + 512 * (ci + 1))
                    psl = slice(512 * ci, 512 * (ci + 1))
                    for ki in range(4):
                        nc.tensor.matmul(
                            ps8[:, psl],
                            w8T[ki][:, 128 * og:128 * (og + 1)],
                            h7b[ki][:, sl],
                            start=(ki == 0), stop=(ki == 3))
                nc.scalar.activation(
                    u8[:, HB * h:HB * (h + 1)], ps8[:, 0:HB], AF.Identity,
                    accum_out=hs2[:, h:h + 1])
            head_sumsq(u8[:], hq2)
            s, b = head_stats_from(hs2, hq2)
            t = hb_pool.tile([128, N], BF16, tag=f"h8b{og}")
            nc.scalar.activation(t[:], u8[:], AF.Prelu, bias=b[:, :], scale=s[:, :], alpha=0.2)
            h8b.append(t)

        o2 = hu_pool.tile([2, N], F32, tag="uh", name="o2")
        for ci in range(8):
            sl = slice(512 * ci, 512 * (ci + 1))
            ps9 = pp.tile([128, HB], F32, tag="pp")
            for ki in range(2):
                nc.tensor.matmul(
                    ps9[0:2, 0:512],
                    w9T[ki][:], h8b[ki][:, sl],
                    start=(ki == 0), stop=(ki == 1))
            nc.scalar.copy(o2[:, sl], ps9[0:2, 0:512])

        ost = hb_pool.tile([128, 64], F32, tag="ost")
        pst = pp.tile([128, HB], F32, tag="pp")
        for t in range(NT):
            nc.tensor.transpose(
                pst[:, 2 * t:2 * (t + 1)], o2[:, 128 * t:128 * (t + 1)], id2[:])
        nc.scalar.copy(ost[:], pst[:, 0:64])
        nc.sync.dma_start(
            out_d.rearrange("(t p) c -> p t c", p=128),
            ost[:].rearrange("p (t c) -> p t c", c=2))

    nc.finalize()
    return nc


def _shared_inputs(ws):
    import ml_dtypes
    w1, w2, w3, w4, w5, w6, w7, w8, w9 = ws
    f32 = np.float32
    bf16 = ml_dtypes.bfloat16
    d = {}
    for bi, w in [(0, w1), (1, w3), (2, w5)]:
        C = w.shape[1] // 2
        waT = np.ascontiguousarray(w[:, :C].T.astype(f32))          # [C, 64]
        wbT = np.ascontiguousarray((w[:, C:] - w[:, :C]).T.astype(f32))
        for h in range(2):
            blk = np.zeros((128, 128), dtype=f32)
            blk[64 * h:64 * h + C, 0:64] = waT
            blk[64 * h:64 * h + C, 64:128] = waT
            d[f"waD{bi}_h{h}"] = blk
        blk = np.zeros((128, 128), dtype=f32)
        blk[0:C, 0:64] = wbT
        blk[64:64 + C, 64:128] = wbT
        d[f"wbD{bi}"] = blk
    for bi, w in [(0, w2), (1, w4)]:
        wT = np.ascontiguousarray(w.T.astype(f32))                  # [64, 64]
        blk = np.zeros((128, 128), dtype=f32)
        blk[0:64, 0:64] = wT
        blk[64:128, 64:128] = wT
        d[f"wcD{bi}"] = blk
        d["w2T" if bi == 0 else "w4T"] = wT
    w6t = w6.T.astype(bf16); w7gt = w7[:, :1024].T.astype(bf16)
    w7xt = w7[:, 1024:].T.astype(bf16); w8t = w8.T.astype(bf16)
    w9t = w9.T.astype(bf16)
    for k in range(3):
        d[f"w6T_{k}"] = np.ascontiguousarray(w6t[64 * k:64 * (k + 1)])
        d[f"w7xT_{k}"] = np.ascontiguousarray(w7xt[64 * k:64 * (k + 1)])
    d["w6T12"] = np.ascontiguousarray(w6t[0:128])
    d["w7xT12"] = np.ascontiguousarray(w7xt[0:128])
    for k in range(8):
        d[f"w7gT_{k}"] = np.ascontiguousarray(w7gt[128 * k:128 * (k + 1)])
    for k in range(4):
        d[f"w8T_{k}"] = np.ascontiguousarray(w8t[128 * k:128 * (k + 1)])
    for k in range(2):
        d[f"w9T_{k}"] = np.ascontiguousarray(w9t[128 * k:128 * (k + 1)])
    d["iota512"] = np.broadcast_to(
        np.arange(CHUNK, dtype=np.uint32)[None, :], (128, CHUNK)).copy()
    d["maskc"] = np.full((128, 1), 0xFFFFFF00, dtype=np.uint32)
    d["magic"] = np.full((128, 1), 0x5F3759DF, dtype=np.uint32)
    d["id2"] = np.eye(2, dtype=f32)
    return d


def _run(inputs, want_debug=False):
    from concourse.bass_utils import run_bass_kernel_spmd

    if "nc" not in _CACHE:
        _CACHE["nc"] = _build_program()
    nc = _CACHE["nc"]

    x = np.asarray(inputs["x"], dtype=np.float32)
    ws = [np.asarray(inputs[f"w{i}"], dtype=np.float32) for i in range(1, 10)]
    shared = _shared_inputs(ws)
    in_maps = []
    for c in range(8):
        m = dict(shared)
        m["xt"] = np.ascontiguousarray(x[c % 4].T.astype(np.float32))
        in_maps.append(m)
    res = run_bass_kernel_spmd(nc, in_maps, list(range(8)))
    out = np.stack([res.results[c]["out"] for c in range(4)])
    if want_debug:
        return out, [res.results[c] for c in range(4)]
    return out


def kernel(**inputs):
    return _run(inputs)
